# revision 8
# baseline (speedup 1.0000x reference)
"""Trainium2 Bass kernel for the DTC vq_codebook problem.

reference math (alpha=1):
    l = x @ W + b                                   [N, 128]
    D = ||l||^2 + ||c||^2 - 2 l @ c.T               [N, 512]
    num = 1 / (1 + D)
    Q = num / num.sum(1)                            [N, 512]
    Fq = Q.sum(0)                                   [512]  (global over N!)
    num2 = Q^2 / Fq   (== num^2 / (s^2 Fq); the s^2 cancels in P)
    P = num2 / num2.sum(1)                          [N, 512]

Distribution: data-parallel over the N axis on 8 cores; W/b/centroids
replicated; Fq all-reduced across cores on device.

Device layout (per core, N_SH = 8192 rows):
    xT shard [1024, 8192] (host-transposed), lT computed as W.T @ xT so no
    on-device transpose of x is needed.  v = 1 + D is built entirely in PSUM
    by the PE: the cdist matmul (lhsT = lT tile, rhs = -2 c.T) plus a K=2
    broadcast matmul adding lnorm[i] (row built from ones.T @ lT^2) and
    1 + cnorm[k].  num = 1/v stays resident in SBUF for phase 2.
"""

import sys

sys.path.insert(0, "/opt/trn_rl_repo")

import numpy as np

import concourse.bass as bass  # noqa: F401  (engine types referenced via nc)
import concourse.tile as tile
from concourse import bacc, mybir, bass_utils
from concourse.bass_interp import get_hw_module

NCORES = 8
N_FULL = 65536
D_IN = 1024
D_LAT = 128
K = 512
N_SH = N_FULL // NCORES  # 8192 rows per core

BLK = 256  # rows per phase-1 block
NBLK = N_SH // BLK  # 32
TPB = BLK // 128  # tiles per block (2)
NTILES = N_SH // 128  # 64
KC = D_IN // 128  # 8 contraction chunks for mm1

F32 = mybir.dt.float32
F32R = mybir.dt.float32r
AF = mybir.ActivationFunctionType
ALU = mybir.AluOpType

_CACHE = {}


def _build(ncores=NCORES, with_cc=True, compile_neff=True):
    nc = bacc.Bacc(
        "TRN2",
        target_bir_lowering=False,
        debug=False,
        enable_asserts=False,
        num_devices=ncores,
    )
    xt_d = nc.dram_tensor("xt", [D_IN, N_SH], F32R, kind="ExternalInput").ap()
    w_d = nc.dram_tensor("w", [D_IN, D_LAT], F32R, kind="ExternalInput").ap()
    cb2_d = nc.dram_tensor("cb2", [D_LAT, K], F32R, kind="ExternalInput").ap()
    ocn_d = nc.dram_tensor("ocn", [2, K], F32R, kind="ExternalInput").ap()
    onec_d = nc.dram_tensor("onec", [128, 1], F32R, kind="ExternalInput").ap()
    bvec_d = nc.dram_tensor("bvec", [128, 1], F32, kind="ExternalInput").ap()

    lt_d = nc.dram_tensor("lt", [D_LAT, N_SH], F32, kind="ExternalOutput").ap()
    q_d = nc.dram_tensor("q", [N_SH, K], F32, kind="ExternalOutput").ap()
    p_d = nc.dram_tensor("p", [N_SH, K], F32, kind="ExternalOutput").ap()

    with tile.TileContext(nc) as tc:
        with (
            tc.tile_pool(name="const", bufs=1) as const,
            tc.tile_pool(name="numpool", bufs=1) as numpool,
            tc.tile_pool(name="xtp", bufs=2) as xtp,
            tc.tile_pool(name="ltp", bufs=2) as ltp,
            tc.tile_pool(name="qp", bufs=3) as qp,
            tc.tile_pool(name="pp", bufs=3) as pp,
            tc.tile_pool(name="small", bufs=4) as small,
            tc.tile_pool(name="ps_mm1", bufs=2, space="PSUM") as ps_mm1,
            tc.tile_pool(name="ps_v", bufs=2, space="PSUM") as ps_v,
            tc.tile_pool(name="ps_ln", bufs=2, space="PSUM") as ps_ln,
            tc.tile_pool(name="ps_fq", bufs=1, space="PSUM") as ps_fq,
            tc.tile_pool(name="dr", bufs=1, space="DRAM") as dr,
        ):
            # ---- constants ----
            w_sb = const.tile([128, KC, D_LAT], F32R)
            for c in range(KC):
                nc.sync.dma_start(out=w_sb[:, c, :], in_=w_d[c * 128 : (c + 1) * 128, :])
            cb2_sb = const.tile([D_LAT, K], F32R)
            nc.sync.dma_start(out=cb2_sb, in_=cb2_d)
            ocn_sb = const.tile([2, K], F32R)
            nc.sync.dma_start(out=ocn_sb, in_=ocn_d)
            onec_sb = const.tile([128, 1], F32R)
            nc.sync.dma_start(out=onec_sb, in_=onec_d)
            bvec_sb = const.tile([128, 1], F32)
            nc.sync.dma_start(out=bvec_sb, in_=bvec_d)

            # resident num = 1/(1+D) for all 64 tiles  (128KB/partition)
            num_all = numpool.tile([128, NTILES, K], F32)

            fq_ps = ps_fq.tile([1, K], F32)

            # ---------------- phase 1 ----------------
            for blk in range(NBLK):
                r0 = blk * BLK
                xt_blk = xtp.tile([128, KC, BLK], F32R)
                nc.sync.dma_start(
                    out=xt_blk,
                    in_=xt_d[:, r0 : r0 + BLK].rearrange("(c p) j -> p c j", p=128),
                )
                lps = ps_mm1.tile([D_LAT, BLK], F32)
                for c in range(KC):
                    nc.tensor.matmul(
                        lps,
                        w_sb[:, c, :],
                        xt_blk[:, c, :],
                        start=(c == 0),
                        stop=(c == KC - 1),
                    )
                # lT block (+bias) rounded to f32r; also the l output
                lt_blk = ltp.tile([D_LAT, BLK], F32R)
                nc.scalar.activation(lt_blk, lps, AF.Identity, bias=bvec_sb)
                nc.sync.dma_start(
                    out=lt_d[:, r0 : r0 + BLK], in_=lt_blk.bitcast(F32)
                )
                # lnorm row: ones.T @ lT^2
                lsq = ltp.tile([D_LAT, BLK], F32R, tag="lsq")
                nc.vector.tensor_mul(lsq, lt_blk.bitcast(F32), lt_blk.bitcast(F32))
                lnps = ps_ln.tile([1, BLK], F32)
                nc.tensor.matmul(lnps, onec_sb, lsq, start=True, stop=True)
                # ln2 = [lnorm_row ; ones_row]  (K=2 stationary for the bcast mm)
                ln2 = ltp.tile([2, BLK], F32R, tag="ln2")
                nc.scalar.activation(ln2[0:1, :], lnps, AF.Copy)
                nc.sync.dma_start(out=ln2[1:2, :], in_=ocn_d[0:1, 0:BLK])

                for t in range(TPB):
                    g = blk * TPB + t
                    v_ps = ps_v.tile([128, K], F32)
                    nc.tensor.matmul(
                        v_ps,
                        lt_blk[:, t * 128 : (t + 1) * 128],
                        cb2_sb,
                        start=True,
                        stop=False,
                    )
                    nc.tensor.matmul(
                        v_ps,
                        ln2[:, t * 128 : (t + 1) * 128],
                        ocn_sb,
                        start=False,
                        stop=True,
                    )
                    num_g = num_all[:, g, :]
                    nc.vector.reciprocal_approx_fast(out=num_g, in_=v_ps)
                    q_tile = qp.tile([128, K], F32R)
                    s_t = small.tile([128, 1], F32, tag="s")
                    # copy exists only for the row-sum accumulator; q_tile is
                    # overwritten by the scaled copy right after
                    nc.scalar.activation(q_tile, num_g, AF.Copy, accum_out=s_t)
                    invs = small.tile([128, 1], F32, tag="invs")
                    nc.vector.reciprocal_approx_fast(out=invs, in_=s_t)
                    nc.vector.tensor_scalar(
                        out=q_tile,
                        in0=num_g,
                        scalar1=invs,
                        scalar2=None,
                        op0=ALU.mult,
                    )
                    nc.tensor.matmul(
                        fq_ps,
                        onec_sb,
                        q_tile,
                        start=(g == 0),
                        stop=(g == NTILES - 1),
                    )
                    nc.sync.dma_start(
                        out=q_d[r0 + t * 128 : r0 + (t + 1) * 128, :],
                        in_=q_tile.bitcast(F32),
                    )

            # ---------------- Fq all-reduce ----------------
            fq_sb = const.tile([1, K], F32)
            nc.scalar.activation(fq_sb, fq_ps, AF.Copy)
            cc_in = dr.tile([1, K], F32)
            cc_out = dr.tile([1, K], F32)
            nc.sync.dma_start(out=cc_in, in_=fq_sb)
            if with_cc:
                nc.gpsimd.collective_compute(
                    "AllReduce",
                    ALU.add,
                    replica_groups=[list(range(ncores))],
                    ins=[cc_in.opt()],
                    outs=[cc_out.opt()],
                )
            else:
                nc.sync.dma_start(out=cc_out, in_=cc_in)
            fqB = const.tile([128, K], F32)
            nc.sync.dma_start(out=fqB, in_=cc_out.partition_broadcast(128).squeeze(1))
            invfqB = const.tile([128, K], F32)
            nc.vector.reciprocal_approx_fast(out=invfqB, in_=fqB)

            # ---------------- phase 2 ----------------
            for g in range(NTILES):
                num_g = num_all[:, g, :]
                sq = pp.tile([128, K], F32, tag="sq")
                nc.scalar.activation(sq, num_g, AF.Square)
                m = pp.tile([128, K], F32, tag="m")
                nc.vector.tensor_mul(m, sq, invfqB)
                p_tile = pp.tile([128, K], F32, tag="p")
                s2 = small.tile([128, 1], F32, tag="s2")
                if g % 2 == 0:
                    # accumulator-only copy (p_tile overwritten below)
                    nc.scalar.activation(p_tile, m, AF.Copy, accum_out=s2)
                else:
                    nc.vector.tensor_reduce(
                        out=s2, in_=m, axis=mybir.AxisListType.X, op=ALU.add
                    )
                invs2 = small.tile([128, 1], F32, tag="invs2")
                nc.vector.reciprocal_approx_fast(out=invs2, in_=s2)
                nc.vector.tensor_scalar(
                    out=p_tile, in0=m, scalar1=invs2, scalar2=None, op0=ALU.mult
                )
                nc.sync.dma_start(
                    out=p_d[g * 128 : (g + 1) * 128, :], in_=p_tile
                )

    nc.compile()
    nc.m = get_hw_module(nc.m)
    return nc


def _make_exec(nc, ncores=NCORES):
    """Build a reusable jitted executor for the compiled module (mirrors
    bass2jax.run_bass_via_pjrt's multi-core path, minus donation so the
    callable can be invoked repeatedly for steady-state timing)."""
    import jax
    from jax.sharding import Mesh, PartitionSpec
    from jax.experimental.shard_map import shard_map
    from concourse import bass2jax as b2j

    b2j.install_neuronx_cc_hook()

    partition_name = nc.partition_id_tensor.name if nc.partition_id_tensor else None
    in_names, out_names, out_avals, zero_outs = [], [], [], []
    for alloc in nc.m.functions[0].allocations:
        if not isinstance(alloc, mybir.MemoryLocationSet):
            continue
        name = alloc.memorylocations[0].name
        if alloc.kind == "ExternalInput":
            if name != partition_name:
                in_names.append(name)
        elif alloc.kind == "ExternalOutput":
            out_names.append(name)
            shape = tuple(alloc.tensor_shape)
            dtype = mybir.dt.np(alloc.dtype)
            out_avals.append(jax.core.ShapedArray(shape, dtype))
            zero_outs.append(np.zeros(shape, dtype))
    n_params = len(in_names)
    all_in_names = in_names + out_names
    if partition_name is not None:
        all_in_names = all_in_names + [partition_name]

    def _body(*args):
        operands = list(args)
        if partition_name is not None:
            operands.append(b2j.partition_id_tensor())
        outs = b2j._bass_exec_p.bind(
            *operands,
            out_avals=tuple(out_avals),
            in_names=tuple(all_in_names),
            out_names=tuple(out_names),
            lowering_input_output_aliases=(),
            sim_require_finite=True,
            sim_require_nnan=True,
            nc=nc,
        )
        return tuple(outs)

    devices = jax.devices()[:ncores]
    mesh = Mesh(np.asarray(devices), ("core",))
    nin = n_params + len(out_names)
    sharded = jax.jit(
        shard_map(
            _body,
            mesh=mesh,
            in_specs=(PartitionSpec("core"),) * nin,
            out_specs=(PartitionSpec("core"),) * len(out_names),
            check_rep=False,
        ),
        keep_unused=True,
    )

    def pack(in_maps):
        concat_in = [
            np.concatenate([np.asarray(in_maps[c][nm]) for c in range(ncores)], axis=0)
            for nm in in_names
        ]
        concat_zeros = [
            np.zeros((ncores * z.shape[0], *z.shape[1:]), z.dtype) for z in zero_outs
        ]
        return [jax.device_put(a) for a in concat_in + concat_zeros]

    def unpack(out_arrs):
        return [
            {
                nm: np.asarray(out_arrs[i]).reshape(ncores, *out_avals[i].shape)[c]
                for i, nm in enumerate(out_names)
            }
            for c in range(ncores)
        ]

    return sharded, pack, unpack


def _prep_in_maps(x, W, b, centroids):
    x = np.ascontiguousarray(x, dtype=np.float32)
    W = np.ascontiguousarray(W, dtype=np.float32)
    b = np.asarray(b, dtype=np.float32)
    centroids = np.ascontiguousarray(centroids, dtype=np.float32)

    xt = np.ascontiguousarray(x.T)  # [D_IN, N]
    cb2 = np.ascontiguousarray(-2.0 * centroids.T)  # [128, 512]
    cn1 = 1.0 + (centroids.astype(np.float64) ** 2).sum(1).astype(np.float32)
    ocn = np.stack([np.ones(K, np.float32), cn1])  # [2, 512]
    onec = np.ones((128, 1), np.float32)
    bvec = b.reshape(D_LAT, 1)

    in_maps = []
    for c in range(NCORES):
        in_maps.append(
            {
                "xt": np.ascontiguousarray(xt[:, c * N_SH : (c + 1) * N_SH]),
                "w": W,
                "cb2": cb2,
                "ocn": ocn,
                "onec": onec,
                "bvec": bvec,
            }
        )
    return in_maps


def _get_exec():
    if "exec" not in _CACHE:
        nc = _build()
        _CACHE["exec"] = _make_exec(nc)
    return _CACHE["exec"]


def kernel(x, W, b, centroids):
    fn, pack, unpack = _get_exec()
    in_maps = _prep_in_maps(x, W, b, centroids)
    args = pack(in_maps)
    results = unpack(fn(*args))
    l = np.empty((N_FULL, D_LAT), np.float32)
    Q = np.empty((N_FULL, K), np.float32)
    P = np.empty((N_FULL, K), np.float32)
    for c in range(NCORES):
        out = results[c]
        l[c * N_SH : (c + 1) * N_SH] = out["lt"].T
        Q[c * N_SH : (c + 1) * N_SH] = out["q"]
        P[c * N_SH : (c + 1) * N_SH] = out["p"]
    return l, Q, P


# revision 24
# speedup vs baseline: 1.5211x; 1.5211x over previous
"""Trainium2 Bass kernel for the DTC vq_codebook problem.

reference math (alpha=1):
    l = x @ W + b                                   [N, 128]
    D = ||l||^2 + ||c||^2 - 2 l @ c.T               [N, 512]
    num = 1 / (1 + D)
    Q = num / num.sum(1)                            [N, 512]
    Fq = Q.sum(0)                                   [512]  (global over N!)
    num2 = Q^2 / Fq   (== num^2 / (s^2 Fq); the s^2 cancels in P)
    P = num2 / num2.sum(1)                          [N, 512]

Distribution: data-parallel over the N axis on 8 cores; W/b/centroids
replicated; Fq all-reduced across cores on device.

Per-core layout (N_SH = 8192 rows):
    xT shard [1024, 8192] (host-transposed, bf16) so lT = W.T @ xT needs no
    on-device transpose.  v = 1 + D is built entirely in PSUM by the PE: the
    cdist matmul (lhsT = lT tile, rhs = -2 c.T, fp32r at full PE rate) plus a
    K=2 broadcast matmul adding lnorm[i] (row from ones.T @ lT^2) and
    1 + cnorm[k].  num = 1/v stays resident in SBUF for phase 2.
    Fq column sums run on the PE (ones.T @ Q) a few tiles behind the main
    pipeline so they never stall the next block's matmuls.
"""

import sys

sys.path.insert(0, "/opt/trn_rl_repo")

import numpy as np

import concourse.tile as tile
from concourse import bacc, mybir, bass_utils  # noqa: F401
from concourse.bass_interp import get_hw_module

NCORES = 8
N_FULL = 65536
D_IN = 1024
D_LAT = 128
K = 512
N_SH = N_FULL // NCORES  # 8192 rows per core

BLK = 512  # rows per phase-1 block
NBLK = N_SH // BLK
TPB = BLK // 128  # tiles per block
NTILES = N_SH // 128  # 64
KC = D_IN // 128  # contraction chunks for mm1

F32 = mybir.dt.float32
F32R = mybir.dt.float32r
BF16 = mybir.dt.bfloat16
AF = mybir.ActivationFunctionType
ALU = mybir.AluOpType

_CACHE = {}


def _build(ncores=NCORES, with_cc=True):
    nc = bacc.Bacc(
        "TRN2",
        target_bir_lowering=False,
        debug=False,
        enable_asserts=False,
        num_devices=ncores,
    )
    xt_d = nc.dram_tensor("xt", [D_IN, N_SH], BF16, kind="ExternalInput").ap()
    w_d = nc.dram_tensor("w", [D_IN, D_LAT], BF16, kind="ExternalInput").ap()
    cb2_d = nc.dram_tensor("cb2", [D_LAT, K], F32R, kind="ExternalInput").ap()
    ocn_d = nc.dram_tensor("ocn", [2, K], F32R, kind="ExternalInput").ap()
    onec_d = nc.dram_tensor("onec", [128, 1], F32R, kind="ExternalInput").ap()
    bvec_d = nc.dram_tensor("bvec", [128, 1], F32, kind="ExternalInput").ap()

    lt_d = nc.dram_tensor("lt", [D_LAT, N_SH], F32, kind="ExternalOutput").ap()
    q_d = nc.dram_tensor("q", [N_SH, K], F32, kind="ExternalOutput").ap()
    p_d = nc.dram_tensor("p", [N_SH, K], F32, kind="ExternalOutput").ap()

    with tile.TileContext(nc) as tc:
        with (
            tc.tile_pool(name="const", bufs=1) as const,
            tc.tile_pool(name="numpool", bufs=1) as numpool,
            tc.tile_pool(name="xtp", bufs=2) as xtp,
            tc.tile_pool(name="ltp", bufs=3) as ltp,
            tc.tile_pool(name="qp", bufs=2) as qp,
            tc.tile_pool(name="pp", bufs=2) as pp,
            tc.tile_pool(name="small", bufs=8) as small,
            tc.tile_pool(name="ps_mm1", bufs=2, space="PSUM") as ps_mm1,
            tc.tile_pool(name="ps_v", bufs=3, space="PSUM") as ps_v,
            tc.tile_pool(name="ps_ln", bufs=2, space="PSUM") as ps_ln,
            tc.tile_pool(name="ps_fq", bufs=1, space="PSUM") as ps_fq,
            tc.tile_pool(name="dr", bufs=1, space="DRAM") as dr,
        ):
            # ---- constants ----
            w_sb = const.tile([128, KC, D_LAT], BF16)
            for c in range(KC):
                nc.sync.dma_start(out=w_sb[:, c, :], in_=w_d[c * 128 : (c + 1) * 128, :])
            cb2_sb = const.tile([D_LAT, K], F32R)
            nc.sync.dma_start(out=cb2_sb, in_=cb2_d)
            ocn_sb = const.tile([2, K], F32R)
            nc.sync.dma_start(out=ocn_sb, in_=ocn_d)
            onec_sb = const.tile([128, 1], F32R)
            nc.sync.dma_start(out=onec_sb, in_=onec_d)
            bvec_sb = const.tile([128, 1], F32)
            nc.sync.dma_start(out=bvec_sb, in_=bvec_d)

            # resident num = 1/(1+D) for all 64 tiles  (128KB/partition)
            num_all = numpool.tile([128, NTILES, K], F32)

            fq_ps = ps_fq.tile([1, K], F32)

            def emit_fq(qhalf, t2, g):
                nc.tensor.matmul(
                    fq_ps,
                    onec_sb,
                    qhalf[:, t2, :],
                    start=(g == 0),
                    stop=(g == NTILES - 1),
                )

            # ---------------- phase 1 ----------------
            # Block-level software pipeline (strict-FIFO engine queues mean
            # emission order IS execution order per engine, so dependent
            # cross-engine stages are emitted one block / a few tiles apart):
            #   prologue1(b): xt DMA + mm1 -> lps
            #   prologue2(b-1): lT (+bias), lT out-DMA, lT^2, lnorm mm, ln2
            #   tiles(b-2):  per-tile stages, themselves lagged:
            #       A(g): cdist+bcast matmuls -> v, recip -> num
            #       B(g-1): ACT copy+accum -> s
            #       C(g-2): invs, Q = num*invs
            #       D(g-3): Fq matmul (+ Q out-DMA per tile pair)
            lps_q, blk_q = [], []

            def prologue1(b):
                r0 = b * BLK
                xt_blk = xtp.tile([128, KC, BLK], BF16)
                nc.sync.dma_start(
                    out=xt_blk,
                    in_=xt_d[:, r0 : r0 + BLK].rearrange("(c p) j -> p c j", p=128),
                )
                lps = ps_mm1.tile([D_LAT, BLK], F32)
                for c in range(KC):
                    nc.tensor.matmul(
                        lps,
                        w_sb[:, c, :],
                        xt_blk[:, c, :],
                        start=(c == 0),
                        stop=(c == KC - 1),
                    )
                lps_q.append(lps)

            def prologue2(b):
                r0 = b * BLK
                lps = lps_q.pop(0)
                lt_blk = ltp.tile([D_LAT, BLK], F32R, tag="lt")
                nc.scalar.activation(lt_blk, lps, AF.Identity, bias=bvec_sb)
                nc.scalar.dma_start(out=lt_d[:, r0 : r0 + BLK], in_=lt_blk.bitcast(F32))
                lsq = ltp.tile([D_LAT, BLK], F32R, tag="lsq", bufs=2)
                nc.scalar.activation(lsq, lt_blk.bitcast(F32), AF.Square)
                lnps = ps_ln.tile([1, BLK], F32)
                nc.tensor.matmul(lnps, onec_sb, lsq, start=True, stop=True)
                ln2 = ltp.tile([2, BLK], F32R, tag="ln2", bufs=2)
                nc.scalar.activation(ln2[0:1, :], lnps, AF.Copy)
                nc.sync.dma_start(out=ln2[1:2, :], in_=ocn_d[0:1, 0:BLK])
                blk_q.append((b, lt_blk, ln2))

            st_a, st_b, st_c = [], [], []

            def tile_stage_a(b, t, lt_blk, ln2, q_half):
                g = b * TPB + t
                v_ps = ps_v.tile([128, K], F32)
                nc.tensor.matmul(
                    v_ps,
                    lt_blk[:, t * 128 : (t + 1) * 128],
                    cb2_sb,
                    start=True,
                    stop=False,
                )
                nc.tensor.matmul(
                    v_ps,
                    ln2[:, t * 128 : (t + 1) * 128],
                    ocn_sb,
                    start=False,
                    stop=True,
                )
                nc.vector.reciprocal_approx_fast(out=num_all[:, g, :], in_=v_ps)
                st_a.append((b, t, q_half))

            def tile_stage_b():
                b, t, q_half = st_a.pop(0)
                g = b * TPB + t
                q_tile = q_half[:, t % 2, :]
                s_t = small.tile([128, 1], F32, tag="s")
                # copy exists only for the row-sum accumulator; q_tile is
                # overwritten by the scaled copy in stage C
                nc.scalar.activation(q_tile, num_all[:, g, :], AF.Copy, accum_out=s_t)
                st_b.append((b, t, q_half, s_t))

            def tile_stage_c():
                b, t, q_half, s_t = st_b.pop(0)
                g = b * TPB + t
                invs = small.tile([128, 1], F32, tag="invs")
                nc.vector.reciprocal_approx_fast(out=invs, in_=s_t)
                nc.vector.tensor_scalar(
                    out=q_half[:, t % 2, :],
                    in0=num_all[:, g, :],
                    scalar1=invs,
                    scalar2=None,
                    op0=ALU.mult,
                )
                st_c.append((b, t, q_half))

            def tile_stage_d():
                b, t, q_half = st_c.pop(0)
                g = b * TPB + t
                emit_fq(q_half, t % 2, g)
                if t % 2 == 1:
                    rq = b * BLK + (t - 1) * 128
                    nc.sync.dma_start(
                        out=q_d[rq : rq + 256, :].rearrange("(t p) k -> p t k", p=128),
                        in_=q_half.bitcast(F32),
                    )

            for step in range(NBLK + 2):
                if step < NBLK:
                    prologue1(step)
                if step >= 1 and step - 1 < NBLK:
                    prologue2(step - 1)
                if step >= 2:
                    b3 = step - 2
                    _, lt_blk, ln2 = blk_q.pop(0)
                    q_half = None
                    for t in range(TPB):
                        if t % 2 == 0:
                            q_half = qp.tile([128, 2, K], F32R, tag="qhalf")
                        tile_stage_a(b3, t, lt_blk, ln2, q_half)
                        if len(st_a) > 1:
                            tile_stage_b()
                        if len(st_b) > 1:
                            tile_stage_c()
                        if len(st_c) > 1:
                            tile_stage_d()
            while st_a:
                tile_stage_b()
                tile_stage_c()
                tile_stage_d()
            while st_b:
                tile_stage_c()
                tile_stage_d()
            while st_c:
                tile_stage_d()

            # ---------------- Fq all-reduce ----------------
            fq_sb = const.tile([1, K], F32)
            nc.scalar.activation(fq_sb, fq_ps, AF.Copy)
            cc_in = dr.tile([1, K], F32)
            cc_out = dr.tile([1, K], F32)
            nc.sync.dma_start(out=cc_in, in_=fq_sb)
            if with_cc:
                nc.gpsimd.collective_compute(
                    "AllReduce",
                    ALU.add,
                    replica_groups=[list(range(ncores))],
                    ins=[cc_in.opt()],
                    outs=[cc_out.opt()],
                )
            else:
                nc.sync.dma_start(out=cc_out, in_=cc_in)
            # rsqB = 1/sqrt(Fq) broadcast, refined by one Newton step (the ACT
            # Sqrt table is low-precision; recip_fast is ~18 bit)
            fqB = const.tile([128, K], F32)
            nc.sync.dma_start(out=fqB, in_=cc_out.partition_broadcast(128).squeeze(1))
            invfqB = const.tile([128, K], F32)
            nc.vector.reciprocal_approx_fast(out=invfqB, in_=fqB)
            rs0 = ps_v.tile([128, K], F32)
            nc.scalar.activation(rs0, invfqB, AF.Sqrt)
            nt1 = ps_v.tile([128, K], F32)
            nc.vector.tensor_mul(nt1, rs0, rs0)
            nc.vector.tensor_mul(nt1, nt1, fqB)
            nc.vector.tensor_scalar(
                out=nt1,
                in0=nt1,
                scalar1=-0.5,
                scalar2=1.5,
                op0=ALU.mult,
                op1=ALU.add,
            )
            rsqB = const.tile([128, 2, K], F32)
            nc.vector.tensor_mul(rsqB[:, 0, :], rs0, nt1)
            nc.vector.tensor_copy(rsqB[:, 1, :], rsqB[:, 0, :])

            # ---------------- phase 2 ----------------
            # m = (num * rsqB)^2 == num^2/Fq; the Square's accum_out gives the
            # row-sum for free.  Tile pairs share the big DVE ops; stages are
            # emission-lagged as in phase 1:
            #   A(j): w = num*rsqB (pair)   B(j-1): m = w^2 (+s2, per tile)
            #   C(j-2): invs2 (pair), P (per tile) + DMA
            p2a, p2b = [], []
            NPAIR = NTILES // 2

            def p2_stage_a(j):
                w_t = pp.tile([128, 2, K], F32, tag="w")
                nc.vector.tensor_mul(w_t, num_all[:, 2 * j : 2 * j + 2, :], rsqB)
                p2a.append((j, w_t))

            def p2_stage_b():
                j, w_t = p2a.pop(0)
                m = pp.tile([128, 2, K], F32, tag="m")
                s2 = small.tile([128, 2], F32, tag="s2")
                nc.scalar.activation(m[:, 0, :], w_t[:, 0, :], AF.Square, accum_out=s2[:, 0:1])
                nc.scalar.activation(m[:, 1, :], w_t[:, 1, :], AF.Square, accum_out=s2[:, 1:2])
                p2b.append((j, m, s2))

            def p2_stage_c():
                j, m, s2 = p2b.pop(0)
                p_half = pp.tile([128, 2, K], F32, tag="phalf")
                invs2 = small.tile([128, 2], F32, tag="invs2")
                nc.vector.reciprocal_approx_fast(out=invs2, in_=s2)
                for t in range(2):
                    nc.vector.tensor_scalar(
                        out=p_half[:, t, :],
                        in0=m[:, t, :],
                        scalar1=invs2[:, t : t + 1],
                        scalar2=None,
                        op0=ALU.mult,
                    )
                rp = 2 * j * 128
                nc.sync.dma_start(
                    out=p_d[rp : rp + 256, :].rearrange("(t p) k -> p t k", p=128),
                    in_=p_half,
                )

            for j in range(NPAIR):
                p2_stage_a(j)
                if len(p2a) > 1:
                    p2_stage_b()
                if len(p2b) > 1:
                    p2_stage_c()
            while p2a:
                p2_stage_b()
                p2_stage_c()
            while p2b:
                p2_stage_c()

    nc.compile()
    nc.m = get_hw_module(nc.m)
    return nc


def _make_exec(nc, ncores=NCORES):
    """Build a reusable jitted executor for the compiled module (mirrors
    bass2jax.run_bass_via_pjrt's multi-core path, minus donation so the
    callable can be invoked repeatedly for steady-state timing)."""
    import jax
    from jax.sharding import Mesh, PartitionSpec
    from jax.experimental.shard_map import shard_map
    from concourse import bass2jax as b2j

    b2j.install_neuronx_cc_hook()

    partition_name = nc.partition_id_tensor.name if nc.partition_id_tensor else None
    in_names, out_names, out_avals, zero_outs = [], [], [], []
    for alloc in nc.m.functions[0].allocations:
        if not isinstance(alloc, mybir.MemoryLocationSet):
            continue
        name = alloc.memorylocations[0].name
        if alloc.kind == "ExternalInput":
            if name != partition_name:
                in_names.append(name)
        elif alloc.kind == "ExternalOutput":
            out_names.append(name)
            shape = tuple(alloc.tensor_shape)
            dtype = mybir.dt.np(alloc.dtype)
            out_avals.append(jax.core.ShapedArray(shape, dtype))
            zero_outs.append(np.zeros(shape, dtype))
    n_params = len(in_names)
    all_in_names = in_names + out_names
    if partition_name is not None:
        all_in_names = all_in_names + [partition_name]

    def _body(*args):
        operands = list(args)
        if partition_name is not None:
            operands.append(b2j.partition_id_tensor())
        outs = b2j._bass_exec_p.bind(
            *operands,
            out_avals=tuple(out_avals),
            in_names=tuple(all_in_names),
            out_names=tuple(out_names),
            lowering_input_output_aliases=(),
            sim_require_finite=True,
            sim_require_nnan=True,
            nc=nc,
        )
        return tuple(outs)

    devices = jax.devices()[:ncores]
    mesh = Mesh(np.asarray(devices), ("core",))
    nin = n_params + len(out_names)
    sharded = jax.jit(
        shard_map(
            _body,
            mesh=mesh,
            in_specs=(PartitionSpec("core"),) * nin,
            out_specs=(PartitionSpec("core"),) * len(out_names),
            check_rep=False,
        ),
        keep_unused=True,
    )

    def pack(in_maps):
        from jax.sharding import NamedSharding

        sh = NamedSharding(mesh, PartitionSpec("core"))
        concat_in = [
            np.concatenate([np.asarray(in_maps[c][nm]) for c in range(ncores)], axis=0)
            for nm in in_names
        ]
        concat_zeros = [
            np.zeros((ncores * z.shape[0], *z.shape[1:]), z.dtype) for z in zero_outs
        ]
        return [jax.device_put(a, sh) for a in concat_in + concat_zeros]

    def unpack(out_arrs):
        return [
            {
                nm: np.asarray(out_arrs[i]).reshape(ncores, *out_avals[i].shape)[c]
                for i, nm in enumerate(out_names)
            }
            for c in range(ncores)
        ]

    return sharded, pack, unpack


def _prep_in_maps(x, W, b, centroids):
    import ml_dtypes

    x = np.ascontiguousarray(x, dtype=np.float32)
    W = np.ascontiguousarray(W, dtype=np.float32)
    b = np.asarray(b, dtype=np.float32)
    centroids = np.ascontiguousarray(centroids, dtype=np.float32)

    xt = np.ascontiguousarray(x.T).astype(ml_dtypes.bfloat16)  # [D_IN, N]
    w16 = W.astype(ml_dtypes.bfloat16)
    cb2 = np.ascontiguousarray(-2.0 * centroids.T)  # [128, 512]
    cn1 = 1.0 + (centroids.astype(np.float64) ** 2).sum(1).astype(np.float32)
    ocn = np.stack([np.ones(K, np.float32), cn1])  # [2, 512]
    onec = np.ones((128, 1), np.float32)
    bvec = b.reshape(D_LAT, 1)

    in_maps = []
    for c in range(NCORES):
        in_maps.append(
            {
                "xt": np.ascontiguousarray(xt[:, c * N_SH : (c + 1) * N_SH]),
                "w": w16,
                "cb2": cb2,
                "ocn": ocn,
                "onec": onec,
                "bvec": bvec,
            }
        )
    return in_maps


def _get_exec():
    if "exec" not in _CACHE:
        nc = _build()
        _CACHE["exec"] = _make_exec(nc)
    return _CACHE["exec"]


def kernel(x, W, b, centroids):
    fn, pack, unpack = _get_exec()
    in_maps = _prep_in_maps(x, W, b, centroids)
    args = pack(in_maps)
    results = unpack(fn(*args))
    l = np.empty((N_FULL, D_LAT), np.float32)
    Q = np.empty((N_FULL, K), np.float32)
    P = np.empty((N_FULL, K), np.float32)
    for c in range(NCORES):
        out = results[c]
        l[c * N_SH : (c + 1) * N_SH] = out["lt"].T
        Q[c * N_SH : (c + 1) * N_SH] = out["q"]
        P[c * N_SH : (c + 1) * N_SH] = out["p"]
    return l, Q, P
